# revision 1
# baseline (speedup 1.0000x reference)
"""TF-IDF document model (histogram_binning) on 8 TRN2 NeuronCores.

Algorithm (per core, 64 batch rows, data-parallel over batch):
  For each row b: tf histogram over vocab V=50257 computed as a radix
  one-hot matmul on the PE: vocab index v = hi*393 + lo with hi in
  [0,128), lo in [0,393). For each 128-token chunk of the row,
  A[s,hi] = (hi_s == hi), B[s,lo] = (lo_s == lo) (fp16 one-hots built
  by DVE tensor_scalar is_equal against an iota tile), and
  C[hi,lo] += A^T @ B accumulates the count matrix in PSUM.
  Then T = C * idf2 (idf reshaped [128,393]) with a fused per-partition
  row-sum; n_b = total sum via a ones-matmul; out_row = T / n_b.
Output written as [64, 50304] per core (vocab padded 50257->50304);
host slices/concats to (512, 50257).
"""
import numpy as np

import concourse.bacc as bacc
import concourse.mybir as mybir
from concourse import bass_utils
from concourse.tile import TileContext

B, S, V = 512, 1024, 50257
NC = 8
BL = B // NC          # 64 rows per core
HI, LO = 128, 393     # radix split: v = hi*LO + lo
VP = HI * LO          # 50304 padded vocab
CH = S // 128         # 8 token chunks per row
GROUP = 8             # rows per normalization group

_cache = {}


def _build(repeat: int = 0, feat: str = "full"):
    nc = bacc.Bacc(
        "TRN2",
        target_bir_lowering=False,
        debug=False,
        enable_asserts=False,
        num_devices=NC,
    )
    hif_t = nc.dram_tensor("hif", [128, BL * CH], mybir.dt.float32, kind="ExternalInput")
    lof_t = nc.dram_tensor("lof", [128, BL * CH], mybir.dt.float32, kind="ExternalInput")
    idf2_t = nc.dram_tensor("idf2", [HI, LO], mybir.dt.float32, kind="ExternalInput")
    iota_t = nc.dram_tensor("iota", [128, LO], mybir.dt.float16, kind="ExternalInput")
    onesc_t = nc.dram_tensor("onesc", [128, 1], mybir.dt.float32, kind="ExternalInput")
    onesr_t = nc.dram_tensor("onesr", [1, 128], mybir.dt.float32, kind="ExternalInput")
    # transposed layout: out[p, b*LO+f] = row b, vocab p*LO+f (host unshuffles)
    out_t = nc.dram_tensor("out", [128, BL * LO], mybir.dt.float32, kind="ExternalOutput")
    ovg = out_t.ap().rearrange("p (g c) -> g p c", g=BL // GROUP)

    AF = mybir.ActivationFunctionType
    OP = mybir.AluOpType
    ncols = BL * CH
    GPB = 0

    with TileContext(nc) as tc:
        with (
            tc.tile_pool(name="const", bufs=1) as cpool,
            tc.tile_pool(name="work", bufs=8) as wpool,
            tc.tile_pool(name="ab", bufs=8) as abpool,
            tc.tile_pool(name="tt", bufs=3) as tpool,
            tc.tile_pool(name="ps", bufs=6, space="PSUM") as pspool,
            tc.tile_pool(name="ps2", bufs=1, space="PSUM") as ps2pool,
        ):
            idf2 = cpool.tile([HI, LO], mybir.dt.float32, tag="idf2")
            nc.sync.dma_start(out=idf2[:], in_=idf2_t.ap())
            iota = cpool.tile([128, LO], mybir.dt.float16, tag="iota")
            nc.sync.dma_start(out=iota[:], in_=iota_t.ap())
            onesc = cpool.tile([128, 1], mybir.dt.float32, tag="onesc")
            nc.sync.dma_start(out=onesc[:], in_=onesc_t.ap())
            onesr = cpool.tile([1, 128], mybir.dt.float32, tag="onesr")
            nc.sync.dma_start(out=onesr[:], in_=onesr_t.ap())

            hif = cpool.tile([128, ncols], mybir.dt.float32, tag="hif")
            nc.sync.dma_start(out=hif[:], in_=hif_t.ap())
            lof = cpool.tile([128, ncols], mybir.dt.float32, tag="lof")
            nc.sync.dma_start(out=lof[:], in_=lof_t.ap())


            def main_body(_iv=None):
              for g in range(BL // GROUP):
                  nsums = wpool.tile([128, GROUP], mybir.dt.float32, tag="nsums")
                  Tg = tpool.tile([128, GROUP * LO], mybir.dt.float32, tag="Tg")
                  def emit_stt(rr, CC):
                      nc.vector.scalar_tensor_tensor(
                          out=Tg[:, rr * LO : (rr + 1) * LO],
                          in0=CC[:],
                          scalar=1.0,
                          in1=idf2[:],
                          op0=OP.mult,
                          op1=OP.mult,
                          accum_out=nsums[:, rr : rr + 1],
                      )
                  pending = None  # defer stt(r) past row r+1's A-builds so the
                  # DVE never idles waiting on the PE to finish MM7(r)
                  for r in range(GROUP):
                      row = g * GROUP + r
                      C = pspool.tile([HI, LO], mybir.dt.float32, tag="C")
                      As = []
                      for c in range(CH):
                          col = row * CH + c
                          A = abpool.tile([128, HI], mybir.dt.float16, tag="A", name=f"A_{row}_{c}")
                          nc.vector.tensor_scalar(
                              out=A[:],
                              in0=iota[:, :HI],
                              scalar1=hif[:, col : col + 1],
                              scalar2=None,
                              op0=OP.is_equal,
                          )
                          As.append(A)
                      if pending is not None:
                          emit_stt(*pending)
                          pending = None
                      for c in range(CH):
                          col = row * CH + c
                          Bt = abpool.tile([128, LO], mybir.dt.float16, tag="B")
                          nc.vector.tensor_scalar(
                              out=Bt[:],
                              in0=iota[:],
                              scalar1=lof[:, col : col + 1],
                              scalar2=None,
                              op0=OP.is_equal,
                          )
                          nc.tensor.matmul(
                              out=C[:],
                              lhsT=As[c][:],
                              rhs=Bt[:],
                              start=(c == 0),
                              stop=(c == CH - 1),
                          )
                      if r < GROUP - 1:
                          pending = (r, C)
                      else:
                          emit_stt(r, C)
                  if feat in ("onehots", "nomm", "justpipe"):
                      continue
                  n_ps = ps2pool.tile([1, GROUP], mybir.dt.float32, tag="nps")
                  nc.tensor.matmul(
                      out=n_ps[:], lhsT=onesc[:], rhs=nsums[:], start=True, stop=True
                  )
                  recip = wpool.tile([1, GROUP], mybir.dt.float32, tag="recip")
                  nc.vector.reciprocal(out=recip[:], in_=n_ps[:])
                  rb_ps = ps2pool.tile([128, GROUP], mybir.dt.float32, tag="rbps")
                  nc.tensor.matmul(
                      out=rb_ps[:], lhsT=onesr[:], rhs=recip[:], start=True, stop=True
                  )
                  rb = wpool.tile([128, GROUP], mybir.dt.float32, tag="rb")
                  nc.vector.tensor_copy(out=rb[:], in_=rb_ps[:])
                  for r in range(GROUP):
                      nc.scalar.activation(
                          out=Tg[:, r * LO : (r + 1) * LO],
                          in_=Tg[:, r * LO : (r + 1) * LO],
                          func=AF.Copy,
                          scale=rb[:, r : r + 1],
                      )
                  if feat == "nodma":
                      nc.vector.tensor_copy(out=nsums[:, :1], in_=Tg[:, :1])
                  else:
                      nc.sync.dma_start(out=ovg[g], in_=Tg[:])
            if repeat:
                tc.For_i_unrolled(0, repeat, 1, main_body, max_unroll=1)
            else:
                main_body()
    nc.compile()
    return nc


def _get_nc():
    if "nc" not in _cache:
        _cache["nc"] = _build()
    return _cache["nc"]


def _host_inputs(x: np.ndarray, idf: np.ndarray):
    """Build per-core input maps from the full inputs."""
    idf_pad = np.zeros(VP, dtype=np.float32)
    idf_pad[:V] = np.asarray(idf, dtype=np.float32)
    idf2 = idf_pad.reshape(HI, LO)
    iota = np.broadcast_to(
        np.arange(LO, dtype=np.float16), (128, LO)
    ).copy()

    onesc = np.ones((128, 1), dtype=np.float32)
    onesr = np.ones((1, 128), dtype=np.float32)

    xi = np.asarray(x, dtype=np.int32)  # values < 2**31, safe cast
    hi_all = (xi // LO).astype(np.float32)
    lo_all = (xi % LO).astype(np.float32)
    in_maps = []
    for k in range(NC):
        # layout [128, BL*CH]: element [p, b*CH+c] = v[b, c*128+p]
        def lay(a):
            ac = a[k * BL : (k + 1) * BL]
            return np.ascontiguousarray(
                ac.reshape(BL, CH, 128).transpose(2, 0, 1).reshape(128, BL * CH)
            )
        in_maps.append(
            {
                "hif": lay(hi_all),
                "lof": lay(lo_all),
                "idf2": idf2,
                "iota": iota,
                "onesc": onesc,
                "onesr": onesr,
            }
        )
    return in_maps


def kernel(x: np.ndarray, idf: np.ndarray) -> np.ndarray:
    nc = _get_nc()
    in_maps = _host_inputs(x, idf)
    res = bass_utils.run_bass_kernel_spmd(nc, in_maps, core_ids=list(range(NC)))
    outs = []
    for r in res.results:
        a = r["out"].reshape(128, BL, LO).transpose(1, 0, 2).reshape(BL, VP)
        outs.append(a[:, :V])
    return np.concatenate(outs, axis=0)



# revision 3
# speedup vs baseline: 2.4667x; 2.4667x over previous
"""TF-IDF document model (histogram_binning) on 8 TRN2 NeuronCores.

Data-parallel over batch: 64 rows per core. Per row, the tf histogram over
vocab V=50257 is computed as a radix one-hot matmul on the PE:
v = hi*394 + lo, hi in [0,128), lo in [0,394).

Key structure (vs. the naive per-chunk one-hot kernel):
  - Host sorts each row's tokens by lo. Chunk c (sorted positions
    [128c,128c+128)) then covers a narrow static lo-window [Q[c], Q[c]+W[c])
    (6-sigma order-statistic bounds), so each accumulating matmul streams
    only ~100 output columns instead of 394.
  - All 8 hi one-hots of a row are built by ONE DVE tensor_tensor is_equal
    with a broadcast access pattern (in0 = hif row chunk broadcast along an
    h-major axis), hitting the 2x DVE mode: A_int[p, h*8+c] = (hif[p,c]==h).
    The matmul lhsT reads the per-chunk one-hot via a strided AP.
  - The per-token idf value (host gather idf[x], like the baseline's host
    divmod) rides the lo one-hot build as the tensor_scalar op1 multiplier,
    so no separate (B,V)-sized tf*idf multiply pass exists.
  - The lo one-hot builds are split between the DVE and the otherwise idle
    GPSIMD (Pool) engine.
  - PSUM is cleared by a K=1 zero matmul, the 8 windowed matmuls accumulate,
    and the ACT engine's PSUM->SBUF copy applies the per-row 1/n scale and
    converts to fp16 (halving the output DMA). Host upcasts to fp32.
"""
import numpy as np

import concourse.bacc as bacc
import concourse.mybir as mybir
from concourse import bass_utils
from concourse.tile import TileContext

B, S, V = 512, 1024, 50257
NC = 8
BL = B // NC          # 64 rows per core
HI, LO = 128, 394     # radix split: v = hi*LO + lo
VP = HI * LO          # 50432 padded vocab
CH = S // 128         # 8 sorted 128-token chunks per row
GROUP = 8             # rows per output DMA

# static lo-windows per sorted chunk (6-sigma order-statistic bounds)
QS = [0, 24, 66, 111, 160, 210, 263, 318]
WS = [76, 108, 120, 124, 124, 120, 108, 76]
WMAX = max(WS)
POOL_CHUNKS = (0, 1, 6, 7)   # lo one-hot builds issued on GPSIMD/Pool

_cache = {}


def _build(repeat: int = 0, feat: str = "full"):
    nc = bacc.Bacc(
        "TRN2",
        target_bir_lowering=False,
        debug=False,
        enable_asserts=False,
        num_devices=NC,
    )
    ncols = BL * CH
    hif_t = nc.dram_tensor("hif", [128, ncols], mybir.dt.float16, kind="ExternalInput")
    lof_t = nc.dram_tensor("lof", [128, ncols], mybir.dt.float32, kind="ExternalInput")
    idfv_t = nc.dram_tensor("idfv", [128, ncols], mybir.dt.float32, kind="ExternalInput")
    iotar_t = nc.dram_tensor("iotar", [128, HI * CH], mybir.dt.float16, kind="ExternalInput")
    iotaw_t = nc.dram_tensor("iotaw", [128, WMAX], mybir.dt.float16, kind="ExternalInput")
    onesc_t = nc.dram_tensor("onesc", [128, 1], mybir.dt.float32, kind="ExternalInput")
    onesr_t = nc.dram_tensor("onesr", [1, 128], mybir.dt.float32, kind="ExternalInput")
    zcol_t = nc.dram_tensor("zcol", [1, 128], mybir.dt.float16, kind="ExternalInput")
    zrow_t = nc.dram_tensor("zrow", [1, LO], mybir.dt.float16, kind="ExternalInput")
    # transposed layout: out[p, r*LO+f] = row r, vocab p*LO+f (host unshuffles)
    out_t = nc.dram_tensor("out", [128, BL * LO], mybir.dt.float16, kind="ExternalOutput")
    ovg = out_t.ap().rearrange("p (g c) -> g p c", g=BL // GROUP)

    AF = mybir.ActivationFunctionType
    OP = mybir.AluOpType

    with TileContext(nc) as tc:
        with (
            tc.tile_pool(name="const", bufs=1) as cpool,
            tc.tile_pool(name="aall", bufs=3) as apool,
            tc.tile_pool(name="bt", bufs=24) as bpool,
            tc.tile_pool(name="tt", bufs=3) as tpool,
            tc.tile_pool(name="small", bufs=2) as spool,
            tc.tile_pool(name="ps", bufs=5, space="PSUM") as pspool,
            tc.tile_pool(name="ps2", bufs=1, space="PSUM") as ps2pool,
        ):
            hif = cpool.tile([128, ncols], mybir.dt.float16, tag="hif")
            nc.sync.dma_start(out=hif[:], in_=hif_t.ap())
            lof = cpool.tile([128, ncols], mybir.dt.float32, tag="lof")
            nc.sync.dma_start(out=lof[:], in_=lof_t.ap())
            idfv = cpool.tile([128, ncols], mybir.dt.float32, tag="idfv")
            nc.sync.dma_start(out=idfv[:], in_=idfv_t.ap())
            iotar = cpool.tile([128, HI * CH], mybir.dt.float16, tag="iotar")
            nc.sync.dma_start(out=iotar[:], in_=iotar_t.ap())
            iotaw = cpool.tile([128, WMAX], mybir.dt.float16, tag="iotaw")
            nc.sync.dma_start(out=iotaw[:], in_=iotaw_t.ap())
            onesc = cpool.tile([128, 1], mybir.dt.float32, tag="onesc")
            nc.sync.dma_start(out=onesc[:], in_=onesc_t.ap())
            onesr = cpool.tile([1, 128], mybir.dt.float32, tag="onesr")
            nc.sync.dma_start(out=onesr[:], in_=onesr_t.ap())
            zcol = cpool.tile([1, 128], mybir.dt.float16, tag="zcol")
            nc.sync.dma_start(out=zcol[:], in_=zcol_t.ap())
            zrow = cpool.tile([1, LO], mybir.dt.float16, tag="zrow")
            nc.sync.dma_start(out=zrow[:], in_=zrow_t.ap())

            iotar3 = iotar[:].rearrange("p (h c) -> p h c", c=CH)

            def main_body(_iv=None):
                # --- per-row 1/n: n_r = sum_t idf[x[r,t]] ---
                n_ps = ps2pool.tile([1, ncols], mybir.dt.float32, tag="nps")
                nc.tensor.matmul(out=n_ps[:], lhsT=onesc[:], rhs=idfv[:], start=True, stop=True)
                nsum = spool.tile([1, BL], mybir.dt.float32, tag="nsum")
                nc.vector.tensor_reduce(
                    out=nsum[:],
                    in_=n_ps[:].rearrange("p (r c) -> p r c", c=CH),
                    axis=mybir.AxisListType.X,
                    op=OP.add,
                )
                recip = spool.tile([1, BL], mybir.dt.float32, tag="recip")
                nc.vector.reciprocal(out=recip[:], in_=nsum[:])
                rb_ps = ps2pool.tile([128, BL], mybir.dt.float32, tag="rbps")
                nc.tensor.matmul(out=rb_ps[:], lhsT=onesr[:], rhs=recip[:], start=True, stop=True)
                rb = spool.tile([128, BL], mybir.dt.float32, tag="rb")
                nc.vector.tensor_copy(out=rb[:], in_=rb_ps[:])

                for g in range(BL // GROUP):
                    Tg = tpool.tile([128, GROUP * LO], mybir.dt.float16, tag="Tg")
                    for rr in range(GROUP):
                        r = g * GROUP + rr
                        # fused hi one-hots: A_int[p, h*CH+c] = (hif[p, r*CH+c] == h)
                        Aall = apool.tile([128, HI * CH], mybir.dt.float16, tag="Aall")
                        hif_exp = hif[:, r * CH : (r + 1) * CH].unsqueeze(1).broadcast_to(
                            [128, HI, CH]
                        )
                        nc.vector.tensor_tensor(
                            out=Aall[:].rearrange("p (h c) -> p h c", c=CH),
                            in0=hif_exp,
                            in1=iotar3,
                            op=OP.is_equal,
                        )
                        Aall3 = Aall[:].rearrange("p (h c) -> p c h", c=CH)

                        C = pspool.tile([128, LO], mybir.dt.float32, tag="C")
                        nc.tensor.matmul(out=C[:], lhsT=zcol[:], rhs=zrow[:], start=True, stop=False)
                        for c in range(CH):
                            col = r * CH + c
                            Bt = bpool.tile([128, WS[c]], mybir.dt.float16, tag="B")
                            eng = nc.gpsimd if c in POOL_CHUNKS else nc.vector
                            eng.tensor_scalar(
                                out=Bt[:],
                                in0=iotaw[:, : WS[c]],
                                scalar1=lof[:, col : col + 1],
                                scalar2=idfv[:, col : col + 1],
                                op0=OP.is_equal,
                                op1=OP.mult,
                            )
                            nc.tensor.matmul(
                                out=C[:, QS[c] : QS[c] + WS[c]],
                                lhsT=Aall3[:, c, :],
                                rhs=Bt[:],
                                start=False,
                                stop=(c == CH - 1),
                            )
                        nc.scalar.activation(
                            out=Tg[:, rr * LO : (rr + 1) * LO],
                            in_=C[:],
                            func=AF.Copy,
                            scale=rb[:, r : r + 1],
                        )
                    if feat == "nodma":
                        nc.vector.tensor_copy(out=nsum[:, :1], in_=Tg[:1, :1])
                    else:
                        nc.sync.dma_start(out=ovg[g], in_=Tg[:])

            if repeat:
                tc.For_i_unrolled(0, repeat, 1, main_body, max_unroll=1)
            else:
                main_body()
    nc.compile()
    return nc


def _get_nc():
    if "nc" not in _cache:
        _cache["nc"] = _build()
    return _cache["nc"]


def _host_inputs(x: np.ndarray, idf: np.ndarray):
    """Build per-core input maps from the full inputs."""
    xi = np.asarray(x, dtype=np.int64).astype(np.int32)  # values < 2**31
    idf32 = np.asarray(idf, dtype=np.float32)
    hi_all = (xi // LO).astype(np.int32)
    lo_all = (xi % LO).astype(np.int32)

    # sort each row's tokens by lo so each 128-chunk falls in a narrow window
    order = np.argsort(lo_all, axis=1, kind="stable")
    hi_s = np.take_along_axis(hi_all, order, axis=1)
    lo_s = np.take_along_axis(lo_all, order, axis=1)
    xs = np.take_along_axis(xi, order, axis=1)
    idfv_s = idf32[xs]  # (B, S) fp32, host gather (index prep like hif/lof)

    # per-chunk window-local lo
    qs = np.asarray(QS, dtype=np.int32)
    ws = np.asarray(WS, dtype=np.int32)
    lo_c = lo_s.reshape(B, CH, 128) - qs[None, :, None]
    assert lo_c.min() >= 0 and (lo_c < ws[None, :, None]).all(), "lo window overflow"

    hif = hi_s.astype(np.float16)
    lof = lo_c.reshape(B, S).astype(np.float32)
    idfv = idfv_s.astype(np.float32)

    iotar = np.broadcast_to(
        np.repeat(np.arange(HI, dtype=np.float16), CH)[None, :], (128, HI * CH)
    ).copy()
    iotaw = np.broadcast_to(np.arange(WMAX, dtype=np.float16), (128, WMAX)).copy()
    onesc = np.ones((128, 1), dtype=np.float32)
    onesr = np.ones((1, 128), dtype=np.float32)
    zcol = np.zeros((1, 128), dtype=np.float16)
    zrow = np.zeros((1, LO), dtype=np.float16)

    in_maps = []
    for k in range(NC):
        # layout [128, BL*CH]: element [p, r*CH+c] = token (row r, sorted pos c*128+p)
        def lay(a):
            ac = a[k * BL : (k + 1) * BL]
            return np.ascontiguousarray(
                ac.reshape(BL, CH, 128).transpose(2, 0, 1).reshape(128, BL * CH)
            )
        in_maps.append(
            {
                "hif": lay(hif),
                "lof": lay(lof),
                "idfv": lay(idfv),
                "iotar": iotar,
                "iotaw": iotaw,
                "onesc": onesc,
                "onesr": onesr,
                "zcol": zcol,
                "zrow": zrow,
            }
        )
    return in_maps


def kernel(x: np.ndarray, idf: np.ndarray) -> np.ndarray:
    nc = _get_nc()
    in_maps = _host_inputs(x, idf)
    res = bass_utils.run_bass_kernel_spmd(nc, in_maps, core_ids=list(range(NC)))
    outs = []
    for r in res.results:
        a = r["out"].reshape(128, BL, LO).transpose(1, 0, 2).reshape(BL, VP)
        outs.append(a[:, :V].astype(np.float32))
    return np.concatenate(outs, axis=0)


# revision 14
# speedup vs baseline: 2.6178x; 1.0612x over previous
"""TF-IDF document model (histogram_binning) on 8 TRN2 NeuronCores.

Data-parallel over batch: 64 rows per core. Per row, the tf histogram over
vocab V=50257 is computed as a radix one-hot matmul on the PE:
v = hi*394 + lo, hi in [0,128), lo in [0,394).

Key structure (vs. the naive per-chunk one-hot kernel):
  - Host sorts each row's tokens by lo. Chunk c (sorted positions
    [128c,128c+128)) then covers a narrow static lo-window [Q[c], Q[c]+W[c]),
    so each accumulating matmul streams only ~100 output columns instead of
    394. (Windows are validated against the input; a data-derived build is
    used as fallback.)
  - All 8 hi one-hots of a row are built by ONE DVE tensor_tensor is_equal
    with a broadcast access pattern (in0 = hif row chunk broadcast along an
    h-major axis), hitting the 2x DVE mode: A_int[p, h*8+c] = (hif[p,c]==h).
    The matmul lhsT reads the per-chunk one-hot via a strided AP.
  - The per-token idf value (host gather idf[x], like the baseline's host
    divmod) rides the lo one-hot build as the tensor_scalar op1 multiplier,
    so no separate (B,V)-sized tf*idf multiply pass exists.
  - The lo one-hot builds are split between the DVE and the otherwise idle
    GPSIMD (Pool) engine (alternating 4/3 per row to balance).
  - PSUM is cleared by a K=1 zero matmul, the 8 windowed matmuls accumulate,
    and the ACT engine's PSUM->SBUF copy applies the per-row 1/n scale and
    converts to fp16 (halving the output DMA). Host upcasts to fp32.
  - Inputs are packed into two large DMAs (the cost of a DMA dispatch is
    dominated by fixed HWDGE/SEQ overheads).
"""
import numpy as np

import concourse.bacc as bacc
import concourse.mybir as mybir
from concourse import bass_utils
from concourse.tile import TileContext

B, S, V = 512, 1024, 50257
NC = 8
BL = B // NC          # 64 rows per core
HI, LO = 128, 394     # radix split: v = hi*LO + lo
VP = HI * LO          # 50432 padded vocab
CH = S // 128         # 8 sorted 128-token chunks per row
GROUP = 4             # rows per output DMA

# static lo-windows per sorted chunk (observed data bounds +-8; the host
# prep asserts every token falls inside its window, kernel() falls back to
# a data-derived build if violated)
QS = [0, 29, 72, 120, 170, 216, 269, 322]
WS = [72, 96, 104, 104, 104, 104, 96, 72]

_cache = {}


def _dve_b_chunks(r):
    """Which chunks' lo-builds run on the DVE for row r (rest on Pool)."""
    return (2, 3, 4, 5) if r % 8 < 5 else (3, 4, 5)


def _build(repeat: int = 0, feat: str = "full", qs=None, ws=None):
    QS, WS = (qs or globals()["QS"]), (ws or globals()["WS"])
    WMAX = max(WS)
    nc = bacc.Bacc(
        "TRN2",
        target_bir_lowering=False,
        debug=False,
        enable_asserts=False,
        num_devices=NC,
    )
    ncols = BL * CH
    # packed inputs: pk16 = hif(ncols) ++ iotar(HI*CH) ++ iotaw(WMAX)
    #                pk32 = lof(ncols) ++ idfv(ncols) ++ onesc(1)
    P16 = ncols + HI * CH + WMAX
    P32 = 2 * ncols + 1
    pk16_t = nc.dram_tensor("pk16", [128, P16], mybir.dt.float16, kind="ExternalInput")
    pk32_t = nc.dram_tensor("pk32", [128, P32], mybir.dt.float32, kind="ExternalInput")
    z16_t = nc.dram_tensor("z16", [1, 128 + LO], mybir.dt.float16, kind="ExternalInput")
    ones32_t = nc.dram_tensor("ones32", [1, 128], mybir.dt.float32, kind="ExternalInput")
    # transposed layout: out[p, r*LO+f] = row r, vocab p*LO+f (host unshuffles)
    out_t = nc.dram_tensor("out", [128, BL * LO], mybir.dt.float16, kind="ExternalOutput")
    ovg = out_t.ap().rearrange("p (g c) -> g p c", g=BL // GROUP)

    AF = mybir.ActivationFunctionType
    OP = mybir.AluOpType

    with TileContext(nc) as tc:
        with (
            tc.tile_pool(name="const", bufs=1) as cpool,
            tc.tile_pool(name="aall", bufs=6) as apool,
            tc.tile_pool(name="bt", bufs=32) as bpool,
            tc.tile_pool(name="tt", bufs=6) as tpool,
            tc.tile_pool(name="small", bufs=2) as spool,
            tc.tile_pool(name="ps", bufs=5, space="PSUM") as pspool,
            tc.tile_pool(name="ps2", bufs=1, space="PSUM") as ps2pool,
        ):
            pk16 = cpool.tile([128, P16], mybir.dt.float16, tag="pk16")
            nc.sync.dma_start(out=pk16[:], in_=pk16_t.ap())
            pk32 = cpool.tile([128, P32], mybir.dt.float32, tag="pk32")
            nc.scalar.dma_start(out=pk32[:], in_=pk32_t.ap())
            z16 = cpool.tile([1, 128 + LO], mybir.dt.float16, tag="z16")
            nc.scalar.dma_start(out=z16[:], in_=z16_t.ap())
            ones32 = cpool.tile([1, 128], mybir.dt.float32, tag="ones32")
            nc.scalar.dma_start(out=ones32[:], in_=ones32_t.ap())

            hif = pk16[:, 0:ncols]
            iotar = pk16[:, ncols : ncols + HI * CH]
            iotaw = pk16[:, ncols + HI * CH : ncols + HI * CH + WMAX]
            lof = pk32[:, 0:ncols]
            idfv = pk32[:, ncols : 2 * ncols]
            onesc = pk32[:, 2 * ncols : 2 * ncols + 1]
            zcol = z16[:, 0:128]
            zrow = z16[:, 128 : 128 + LO]
            onesr = ones32[:, :]

            iotar3 = iotar.rearrange("p (h c) -> p h c", c=CH)

            def main_body(_iv=None):
                # --- per-row 1/n: n_r = sum_t idf[x[r,t]] ---
                n_ps = ps2pool.tile([1, ncols], mybir.dt.float32, tag="nps")
                nc.tensor.matmul(out=n_ps[:], lhsT=onesc, rhs=idfv, start=True, stop=True)
                nsum = spool.tile([1, BL], mybir.dt.float32, tag="nsum")
                nc.vector.tensor_reduce(
                    out=nsum[:],
                    in_=n_ps[:].rearrange("p (r c) -> p r c", c=CH),
                    axis=mybir.AxisListType.X,
                    op=OP.add,
                )
                recip = spool.tile([1, BL], mybir.dt.float32, tag="recip")
                nc.vector.reciprocal(out=recip[:], in_=nsum[:])
                rb_ps = ps2pool.tile([128, BL], mybir.dt.float32, tag="rbps")
                nc.tensor.matmul(out=rb_ps[:], lhsT=onesr, rhs=recip[:], start=True, stop=True)
                rb = spool.tile([128, BL], mybir.dt.float32, tag="rb")
                nc.scalar.activation(out=rb[:], in_=rb_ps[:], func=AF.Copy, scale=1.0)

                for g in range(BL // GROUP):
                    Tg = tpool.tile([128, GROUP * LO], mybir.dt.float16, tag="Tg")
                    for rr in range(GROUP):
                        r = g * GROUP + rr
                        dve_chunks = _dve_b_chunks(r)
                        # fused hi one-hots: A_int[p, h*CH+c] = (hif[p, r*CH+c] == h)
                        Aall = apool.tile([128, HI * CH], mybir.dt.float16, tag="Aall")
                        hif_exp = hif[:, r * CH : (r + 1) * CH].unsqueeze(1).broadcast_to(
                            [128, HI, CH]
                        )
                        nc.vector.tensor_tensor(
                            out=Aall[:].rearrange("p (h c) -> p h c", c=CH),
                            in0=hif_exp,
                            in1=iotar3,
                            op=OP.is_equal,
                        )
                        Aall3 = Aall[:].rearrange("p (h c) -> p c h", c=CH)

                        C = pspool.tile([128, LO], mybir.dt.float32, tag="C")
                        nc.tensor.matmul(out=C[:], lhsT=zcol, rhs=zrow, start=True, stop=False)
                        for c in range(CH):
                            col = r * CH + c
                            Bt = bpool.tile([128, WMAX], mybir.dt.float16, tag="B")
                            eng = nc.vector if c in dve_chunks else nc.gpsimd
                            eng.tensor_scalar(
                                out=Bt[:, : WS[c]],
                                in0=iotaw[:, : WS[c]],
                                scalar1=lof[:, col : col + 1],
                                scalar2=idfv[:, col : col + 1],
                                op0=OP.is_equal,
                                op1=OP.mult,
                            )
                            nc.tensor.matmul(
                                out=C[:, QS[c] : QS[c] + WS[c]],
                                lhsT=Aall3[:, c, :],
                                rhs=Bt[:, : WS[c]],
                                start=False,
                                stop=(c == CH - 1),
                            )
                        nc.scalar.activation(
                            out=Tg[:, rr * LO : (rr + 1) * LO],
                            in_=C[:],
                            func=AF.Copy,
                            scale=rb[:, r : r + 1],
                        )
                    if feat == "nodma":
                        nc.vector.tensor_copy(out=nsum[:, :1], in_=Tg[:1, :1])
                    else:
                        nc.sync.dma_start(out=ovg[g], in_=Tg[:])

            if repeat:
                tc.For_i_unrolled(0, repeat, 1, main_body, max_unroll=1)
            else:
                main_body()
    nc.compile()
    return nc


def _get_nc():
    if "nc" not in _cache:
        _cache["nc"] = _build()
    return _cache["nc"]


def _fits(lo_s: np.ndarray, qs, ws) -> bool:
    lo_c = lo_s.reshape(B, CH, 128)
    qa = np.asarray(qs, dtype=np.int32)[None, :, None]
    wa = np.asarray(ws, dtype=np.int32)[None, :, None]
    return bool(((lo_c >= qa) & (lo_c < qa + wa)).all())


def _windows_from_data(lo_s: np.ndarray):
    """Data-derived safe windows (used only if the static ones don't fit)."""
    qs, ws = [], []
    lo_c = lo_s.reshape(B, CH, 128)
    for c in range(CH):
        lo_b = max(0, int(lo_c[:, c].min()) - 8)
        hi_b = min(LO, int(lo_c[:, c].max()) + 1 + 8)
        w = (hi_b - lo_b + 3) // 4 * 4
        if lo_b + w > LO:
            lo_b = LO - w
        qs.append(lo_b)
        ws.append(w)
    return qs, ws


def _host_inputs(x: np.ndarray, idf: np.ndarray, qs=None, ws=None):
    """Build per-core input maps from the full inputs."""
    qs, ws = (qs or QS), (ws or WS)
    wmax = max(ws)
    xi = np.asarray(x, dtype=np.int64).astype(np.int32)  # values < 2**31
    idf32 = np.asarray(idf, dtype=np.float32)
    hi_all = (xi // LO).astype(np.int32)
    lo_all = (xi % LO).astype(np.int32)

    # sort each row's tokens by lo so each 128-chunk falls in a narrow window
    order = np.argsort(lo_all, axis=1, kind="stable")
    hi_s = np.take_along_axis(hi_all, order, axis=1)
    lo_s = np.take_along_axis(lo_all, order, axis=1)
    xs = np.take_along_axis(xi, order, axis=1)
    idfv_s = idf32[xs]  # (B, S) fp32, host gather (index prep like hif/lof)

    # per-chunk window-local lo
    qa = np.asarray(qs, dtype=np.int32)
    wa = np.asarray(ws, dtype=np.int32)
    lo_c = lo_s.reshape(B, CH, 128) - qa[None, :, None]
    assert lo_c.min() >= 0 and (lo_c < wa[None, :, None]).all(), "lo window overflow"

    hif = hi_s.astype(np.float16)
    lof = lo_c.reshape(B, S).astype(np.float32)
    idfv = idfv_s.astype(np.float32)

    iotar = np.repeat(np.arange(HI, dtype=np.float16), CH)
    iotaw = np.arange(wmax, dtype=np.float16)
    consts16 = np.concatenate([iotar, iotaw])  # shared across partitions
    z16 = np.zeros((1, 128 + LO), dtype=np.float16)
    ones32 = np.ones((1, 128), dtype=np.float32)

    ncols = BL * CH
    in_maps = []
    for k in range(NC):
        # layout [128, BL*CH]: element [p, r*CH+c] = token (row r, sorted pos c*128+p)
        def lay(a):
            ac = a[k * BL : (k + 1) * BL]
            return np.ascontiguousarray(
                ac.reshape(BL, CH, 128).transpose(2, 0, 1).reshape(128, BL * CH)
            )
        pk16 = np.empty((128, ncols + len(consts16)), dtype=np.float16)
        pk16[:, :ncols] = lay(hif)
        pk16[:, ncols:] = consts16[None, :]
        pk32 = np.empty((128, 2 * ncols + 1), dtype=np.float32)
        pk32[:, :ncols] = lay(lof)
        pk32[:, ncols : 2 * ncols] = lay(idfv)
        pk32[:, 2 * ncols] = 1.0
        in_maps.append({"pk16": pk16, "pk32": pk32, "z16": z16, "ones32": ones32})
    return in_maps


def kernel(x: np.ndarray, idf: np.ndarray) -> np.ndarray:
    # check the static windows against this input; fall back to data-derived
    # windows (fresh build) if they don't fit
    xi = np.asarray(x, dtype=np.int64).astype(np.int32)
    lo_s = np.sort((xi % LO).astype(np.int32), axis=1)
    if _fits(lo_s, QS, WS):
        nc = _get_nc()
        in_maps = _host_inputs(x, idf)
    else:
        qs, ws = _windows_from_data(lo_s)
        key = ("dyn", tuple(qs), tuple(ws))
        if key not in _cache:
            _cache[key] = _build(qs=qs, ws=ws)
        nc = _cache[key]
        in_maps = _host_inputs(x, idf, qs, ws)
    res = bass_utils.run_bass_kernel_spmd(nc, in_maps, core_ids=list(range(NC)))
    outs = []
    for r in res.results:
        a = r["out"].reshape(128, BL, LO).transpose(1, 0, 2).reshape(BL, VP)
        outs.append(a[:, :V].astype(np.float32))
    return np.concatenate(outs, axis=0)


# revision 15
# speedup vs baseline: 2.6324x; 1.0056x over previous
"""TF-IDF document model (histogram_binning) on 8 TRN2 NeuronCores.

Data-parallel over batch: 64 rows per core. Per row, the tf histogram over
vocab V=50257 is computed as a radix one-hot matmul on the PE:
v = hi*394 + lo, hi in [0,128), lo in [0,394).

Key structure (vs. the naive per-chunk one-hot kernel):
  - Host sorts each row's tokens by lo. Chunk c (sorted positions
    [128c,128c+128)) then covers a narrow static lo-window [Q[c], Q[c]+W[c]),
    so each accumulating matmul streams only ~100 output columns instead of
    394. (Windows are validated against the input; a data-derived build is
    used as fallback.)
  - All 8 hi one-hots of a row are built by ONE DVE tensor_tensor is_equal
    with a broadcast access pattern (in0 = hif row chunk broadcast along an
    h-major axis), hitting the 2x DVE mode: A_int[p, h*8+c] = (hif[p,c]==h).
    The matmul lhsT reads the per-chunk one-hot via a strided AP.
  - The per-token idf value (host gather idf[x], like the baseline's host
    divmod) rides the lo one-hot build as the tensor_scalar op1 multiplier,
    so no separate (B,V)-sized tf*idf multiply pass exists.
  - The lo one-hot builds are split between the DVE and the otherwise idle
    GPSIMD (Pool) engine (alternating 4/3 per row to balance).
  - PSUM is cleared by a K=1 zero matmul, the 8 windowed matmuls accumulate,
    and the ACT engine's PSUM->SBUF copy applies the per-row 1/n scale and
    converts to fp16 (halving the output DMA). Host upcasts to fp32.
  - Inputs are packed into two large DMAs (the cost of a DMA dispatch is
    dominated by fixed HWDGE/SEQ overheads).
"""
import numpy as np

import concourse.bacc as bacc
import concourse.mybir as mybir
from concourse import bass_utils
from concourse.tile import TileContext

B, S, V = 512, 1024, 50257
NC = 8
BL = B // NC          # 64 rows per core
HI, LO = 128, 394     # radix split: v = hi*LO + lo
VP = HI * LO          # 50432 padded vocab
CH = S // 128         # 8 sorted 128-token chunks per row
GROUP = 2             # rows per output DMA

# static lo-windows per sorted chunk (observed data bounds +-8; the host
# prep asserts every token falls inside its window, kernel() falls back to
# a data-derived build if violated)
QS = [0, 29, 72, 120, 170, 216, 269, 322]
WS = [72, 96, 104, 104, 104, 104, 96, 72]

_cache = {}


def _dve_b_chunks(r):
    """Which chunks' lo-builds run on the DVE for row r (rest on Pool)."""
    return (2, 3, 4, 5) if r % 8 < 5 else (3, 4, 5)


def _build(repeat: int = 0, feat: str = "full", qs=None, ws=None):
    QS, WS = (qs or globals()["QS"]), (ws or globals()["WS"])
    WMAX = max(WS)
    nc = bacc.Bacc(
        "TRN2",
        target_bir_lowering=False,
        debug=False,
        enable_asserts=False,
        num_devices=NC,
    )
    ncols = BL * CH
    # packed inputs: hif16 first (smallest, unblocks the DVE), then consts,
    # then pk32 = lof(ncols) ++ idfv(ncols) ++ onesc(1)
    K16 = HI * CH + WMAX
    P32 = 2 * ncols + 1
    hif16_t = nc.dram_tensor("hif16", [128, ncols], mybir.dt.float16, kind="ExternalInput")
    ck16_t = nc.dram_tensor("ck16", [128, K16], mybir.dt.float16, kind="ExternalInput")
    pk32_t = nc.dram_tensor("pk32", [128, P32], mybir.dt.float32, kind="ExternalInput")
    z16_t = nc.dram_tensor("z16", [1, 128 + LO], mybir.dt.float16, kind="ExternalInput")
    ones32_t = nc.dram_tensor("ones32", [1, 128], mybir.dt.float32, kind="ExternalInput")
    # transposed layout: out[p, r*LO+f] = row r, vocab p*LO+f (host unshuffles)
    out_t = nc.dram_tensor("out", [128, BL * LO], mybir.dt.float16, kind="ExternalOutput")
    ovg = out_t.ap().rearrange("p (g c) -> g p c", g=BL // GROUP)

    AF = mybir.ActivationFunctionType
    OP = mybir.AluOpType

    with TileContext(nc) as tc:
        with (
            tc.tile_pool(name="const", bufs=1) as cpool,
            tc.tile_pool(name="aall", bufs=6) as apool,
            tc.tile_pool(name="bt", bufs=32) as bpool,
            tc.tile_pool(name="tt", bufs=6) as tpool,
            tc.tile_pool(name="small", bufs=2) as spool,
            tc.tile_pool(name="ps", bufs=6, space="PSUM") as pspool,
            tc.tile_pool(name="ps2", bufs=1, space="PSUM") as ps2pool,
        ):
            hif16 = cpool.tile([128, ncols], mybir.dt.float16, tag="hif16")
            nc.sync.dma_start(out=hif16[:], in_=hif16_t.ap())
            ck16 = cpool.tile([128, K16], mybir.dt.float16, tag="ck16")
            nc.sync.dma_start(out=ck16[:], in_=ck16_t.ap())
            pk32 = cpool.tile([128, P32], mybir.dt.float32, tag="pk32")
            nc.scalar.dma_start(out=pk32[:], in_=pk32_t.ap())
            z16 = cpool.tile([1, 128 + LO], mybir.dt.float16, tag="z16")
            nc.scalar.dma_start(out=z16[:], in_=z16_t.ap())
            ones32 = cpool.tile([1, 128], mybir.dt.float32, tag="ones32")
            nc.scalar.dma_start(out=ones32[:], in_=ones32_t.ap())

            hif = hif16[:, :]
            iotar = ck16[:, 0 : HI * CH]
            iotaw = ck16[:, HI * CH : HI * CH + WMAX]
            lof = pk32[:, 0:ncols]
            idfv = pk32[:, ncols : 2 * ncols]
            onesc = pk32[:, 2 * ncols : 2 * ncols + 1]
            zcol = z16[:, 0:128]
            zrow = z16[:, 128 : 128 + LO]
            onesr = ones32[:, :]

            iotar3 = iotar.rearrange("p (h c) -> p h c", c=CH)

            def main_body(_iv=None):
                # --- per-row 1/n: n_r = sum_t idf[x[r,t]] ---
                n_ps = ps2pool.tile([1, ncols], mybir.dt.float32, tag="nps")
                nc.tensor.matmul(out=n_ps[:], lhsT=onesc, rhs=idfv, start=True, stop=True)
                nsum = spool.tile([1, BL], mybir.dt.float32, tag="nsum")
                nc.vector.tensor_reduce(
                    out=nsum[:],
                    in_=n_ps[:].rearrange("p (r c) -> p r c", c=CH),
                    axis=mybir.AxisListType.X,
                    op=OP.add,
                )
                recip = spool.tile([1, BL], mybir.dt.float32, tag="recip")
                nc.vector.reciprocal(out=recip[:], in_=nsum[:])
                rb_ps = ps2pool.tile([128, BL], mybir.dt.float32, tag="rbps")
                nc.tensor.matmul(out=rb_ps[:], lhsT=onesr, rhs=recip[:], start=True, stop=True)
                rb = spool.tile([128, BL], mybir.dt.float32, tag="rb")
                nc.scalar.activation(out=rb[:], in_=rb_ps[:], func=AF.Copy, scale=1.0)

                for g in range(BL // GROUP):
                    Tg = tpool.tile([128, GROUP * LO], mybir.dt.float16, tag="Tg")
                    for rr in range(GROUP):
                        r = g * GROUP + rr
                        dve_chunks = _dve_b_chunks(r)
                        # fused hi one-hots: A_int[p, h*CH+c] = (hif[p, r*CH+c] == h)
                        Aall = apool.tile([128, HI * CH], mybir.dt.float16, tag="Aall")
                        hif_exp = hif[:, r * CH : (r + 1) * CH].unsqueeze(1).broadcast_to(
                            [128, HI, CH]
                        )
                        nc.vector.tensor_tensor(
                            out=Aall[:].rearrange("p (h c) -> p h c", c=CH),
                            in0=hif_exp,
                            in1=iotar3,
                            op=OP.is_equal,
                        )
                        Aall3 = Aall[:].rearrange("p (h c) -> p c h", c=CH)

                        C = pspool.tile([128, LO], mybir.dt.float32, tag="C")
                        nc.tensor.matmul(out=C[:], lhsT=zcol, rhs=zrow, start=True, stop=False)
                        for c in range(CH):
                            col = r * CH + c
                            Bt = bpool.tile([128, WMAX], mybir.dt.float16, tag="B")
                            eng = nc.vector if c in dve_chunks else nc.gpsimd
                            eng.tensor_scalar(
                                out=Bt[:, : WS[c]],
                                in0=iotaw[:, : WS[c]],
                                scalar1=lof[:, col : col + 1],
                                scalar2=idfv[:, col : col + 1],
                                op0=OP.is_equal,
                                op1=OP.mult,
                            )
                            nc.tensor.matmul(
                                out=C[:, QS[c] : QS[c] + WS[c]],
                                lhsT=Aall3[:, c, :],
                                rhs=Bt[:, : WS[c]],
                                start=False,
                                stop=(c == CH - 1),
                            )
                        nc.scalar.activation(
                            out=Tg[:, rr * LO : (rr + 1) * LO],
                            in_=C[:],
                            func=AF.Copy,
                            scale=rb[:, r : r + 1],
                        )
                    if feat == "nodma":
                        nc.vector.tensor_copy(out=nsum[:, :1], in_=Tg[:1, :1])
                    else:
                        nc.sync.dma_start(out=ovg[g], in_=Tg[:])

            if repeat:
                tc.For_i_unrolled(0, repeat, 1, main_body, max_unroll=1)
            else:
                main_body()
    nc.compile()
    return nc


def _get_nc():
    if "nc" not in _cache:
        _cache["nc"] = _build()
    return _cache["nc"]


def _fits(lo_s: np.ndarray, qs, ws) -> bool:
    lo_c = lo_s.reshape(B, CH, 128)
    qa = np.asarray(qs, dtype=np.int32)[None, :, None]
    wa = np.asarray(ws, dtype=np.int32)[None, :, None]
    return bool(((lo_c >= qa) & (lo_c < qa + wa)).all())


def _windows_from_data(lo_s: np.ndarray):
    """Data-derived safe windows (used only if the static ones don't fit)."""
    qs, ws = [], []
    lo_c = lo_s.reshape(B, CH, 128)
    for c in range(CH):
        lo_b = max(0, int(lo_c[:, c].min()) - 8)
        hi_b = min(LO, int(lo_c[:, c].max()) + 1 + 8)
        w = (hi_b - lo_b + 3) // 4 * 4
        if lo_b + w > LO:
            lo_b = LO - w
        qs.append(lo_b)
        ws.append(w)
    return qs, ws


def _host_inputs(x: np.ndarray, idf: np.ndarray, qs=None, ws=None):
    """Build per-core input maps from the full inputs."""
    qs, ws = (qs or QS), (ws or WS)
    wmax = max(ws)
    xi = np.asarray(x, dtype=np.int64).astype(np.int32)  # values < 2**31
    idf32 = np.asarray(idf, dtype=np.float32)
    hi_all = (xi // LO).astype(np.int32)
    lo_all = (xi % LO).astype(np.int32)

    # sort each row's tokens by lo so each 128-chunk falls in a narrow window
    order = np.argsort(lo_all, axis=1, kind="stable")
    hi_s = np.take_along_axis(hi_all, order, axis=1)
    lo_s = np.take_along_axis(lo_all, order, axis=1)
    xs = np.take_along_axis(xi, order, axis=1)
    idfv_s = idf32[xs]  # (B, S) fp32, host gather (index prep like hif/lof)

    # per-chunk window-local lo
    qa = np.asarray(qs, dtype=np.int32)
    wa = np.asarray(ws, dtype=np.int32)
    lo_c = lo_s.reshape(B, CH, 128) - qa[None, :, None]
    assert lo_c.min() >= 0 and (lo_c < wa[None, :, None]).all(), "lo window overflow"

    hif = hi_s.astype(np.float16)
    lof = lo_c.reshape(B, S).astype(np.float32)
    idfv = idfv_s.astype(np.float32)

    iotar = np.repeat(np.arange(HI, dtype=np.float16), CH)
    iotaw = np.arange(wmax, dtype=np.float16)
    consts16 = np.concatenate([iotar, iotaw])  # shared across partitions
    z16 = np.zeros((1, 128 + LO), dtype=np.float16)
    ones32 = np.ones((1, 128), dtype=np.float32)

    ncols = BL * CH
    in_maps = []
    for k in range(NC):
        # layout [128, BL*CH]: element [p, r*CH+c] = token (row r, sorted pos c*128+p)
        def lay(a):
            ac = a[k * BL : (k + 1) * BL]
            return np.ascontiguousarray(
                ac.reshape(BL, CH, 128).transpose(2, 0, 1).reshape(128, BL * CH)
            )
        ck16 = np.broadcast_to(consts16[None, :], (128, len(consts16))).copy()
        pk32 = np.empty((128, 2 * ncols + 1), dtype=np.float32)
        pk32[:, :ncols] = lay(lof)
        pk32[:, ncols : 2 * ncols] = lay(idfv)
        pk32[:, 2 * ncols] = 1.0
        in_maps.append({"hif16": lay(hif), "ck16": ck16, "pk32": pk32, "z16": z16, "ones32": ones32})
    return in_maps


def kernel(x: np.ndarray, idf: np.ndarray) -> np.ndarray:
    # check the static windows against this input; fall back to data-derived
    # windows (fresh build) if they don't fit
    xi = np.asarray(x, dtype=np.int64).astype(np.int32)
    lo_s = np.sort((xi % LO).astype(np.int32), axis=1)
    if _fits(lo_s, QS, WS):
        nc = _get_nc()
        in_maps = _host_inputs(x, idf)
    else:
        qs, ws = _windows_from_data(lo_s)
        key = ("dyn", tuple(qs), tuple(ws))
        if key not in _cache:
            _cache[key] = _build(qs=qs, ws=ws)
        nc = _cache[key]
        in_maps = _host_inputs(x, idf, qs, ws)
    res = bass_utils.run_bass_kernel_spmd(nc, in_maps, core_ids=list(range(NC)))
    outs = []
    for r in res.results:
        a = r["out"].reshape(128, BL, LO).transpose(1, 0, 2).reshape(BL, VP)
        outs.append(a[:, :V].astype(np.float32))
    return np.concatenate(outs, axis=0)
